# revision 1
# baseline (speedup 1.0000x reference)
"""Distributed Bass kernel for nn_AttentionCircuit (B=2,S=2048,D=2048,RANK=512,H=16).

Sharding: 8 cores = 2 batches x 4 head-groups (4 heads / 512 D-cols each).
All matmuls in float32r (TF32-like, 4x faster than fp32 on TensorE).

Per-core dataflow (everything laid out so matmul contraction lands on the
partition axis with no on-device transposes; host pre-transposes x / gates):
  A2: t_v^T  = v_read  @ x^T   (stream x^T), gate with g_V^T -> Vg^T
  B2: V      = Vg^T.T @ v_write_hg                       (natural [s,d'])
  A1: t_qk^T = qk_read @ x^T   (stream x^T), gate -> Qg^T, Kg^T
  B1: Q^T/K^T = qk_write_hg.T @ {Q,K}g^T                 (transposed [d',s])
  C:  per si-chunk, per head: scores^T = K^T.T Q^T -> exp (no max-sub; scores
      are small) -> causal mask (block-skip + 4 static masks) -> rowsum via
      ones-matmul -> PV matmul -> normalize w/ 1/(0.81*rowsum)  -> AO^T chunk
      -> AllGather(group of 4) -> D: out_cols = AO_full^T.T @ W_O[:,cols],
      overlapped chunk-wise with C.
"""
import sys
import numpy as np

sys.path.insert(0, '/opt/trn_rl_repo')

import concourse.bass as bass  # noqa: E402
from concourse import bacc  # noqa: E402
import concourse.mybir as mybir  # noqa: E402
import concourse.tile as tile  # noqa: E402
from concourse.bass_utils import run_bass_kernel_spmd  # noqa: E402

B, S, D = 2, 2048, 2048
RANK = 512
NH = 16
HG = 4              # head-groups == cores per batch
DHG = D // HG       # 512 cols per head-group (4 heads)
DH = D // NH        # 128 head dim
P = 128
DB = D // P         # 16 d-blocks
RB = RANK // P      # 4 rank-blocks
SB = S // P         # 16 s-blocks
NT = S // 512       # 4 si tiles of 512
SC = 256            # phase-A s-chunk width
NSC = S // SC       # 8

F32 = mybir.dt.float32
F32R = mybir.dt.float32r
AF = mybir.ActivationFunctionType
ALU = mybir.AluOpType

EXP_SCALE = 1.0 / float(np.sqrt(DH))
INV_KEEP2 = 1.0 / (0.9 * 0.9)
RGROUPS = [[0, 1, 2, 3], [4, 5, 6, 7]]

_CACHE = {}


def _r(ap):
    """[ (o p), f ] DRAM tensor -> [p, o, f] partition-tiled view."""
    return ap.rearrange("(o p) f -> p o f", p=P)


def _build():
    nc = bacc.Bacc("TRN2", target_bir_lowering=False, debug=False,
                   enable_asserts=False, num_devices=8)
    xT = nc.dram_tensor("xT", [D, S], F32, kind="ExternalInput").ap()
    gqT = nc.dram_tensor("gqT", [RANK, S], F32, kind="ExternalInput").ap()
    gkT = nc.dram_tensor("gkT", [RANK, S], F32, kind="ExternalInput").ap()
    gvT = nc.dram_tensor("gvT", [RANK, S], F32, kind="ExternalInput").ap()
    qk_readT = nc.dram_tensor("qk_readT", [D, RANK], F32, kind="ExternalInput").ap()
    v_readT = nc.dram_tensor("v_readT", [D, RANK], F32, kind="ExternalInput").ap()
    qk_w = nc.dram_tensor("qk_write_hg", [RANK, DHG], F32, kind="ExternalInput").ap()
    v_w = nc.dram_tensor("v_write_hg", [RANK, DHG], F32, kind="ExternalInput").ap()
    wo = nc.dram_tensor("wo_cols", [D, DHG], F32, kind="ExternalInput").ap()
    out = nc.dram_tensor("out", [S, DHG], F32, kind="ExternalOutput").ap()

    with tile.TileContext(nc) as tc:
        _body(tc, xT, gqT, gkT, gvT, qk_readT, v_readT, qk_w, v_w, wo, out)
    nc.compile()
    return nc


def _body(tc, xT, gqT, gkT, gvT, qk_readT, v_readT, qk_w, v_w, wo, out):
    nc = tc.nc
    import contextlib
    ctx = contextlib.ExitStack()
    with ctx:
        # ---- long-lived activation tensors (allocated in phase order)
        pool_qk = ctx.enter_context(tc.tile_pool(name="qk", bufs=1))
        QT_sb = pool_qk.tile([P, HG, S], F32R)          # Q^T [d', s]
        KT_sb = pool_qk.tile([P, HG, S], F32R)

        # ========== A1+B1 fused per s-chunk: Q^T, K^T ==========
        with (
            tc.tile_pool(name="qkread", bufs=1) as pool_qr,
            tc.tile_pool(name="qkw", bufs=1) as pool_qw,
            tc.tile_pool(name="ax1", bufs=2) as pool_x,
            tc.tile_pool(name="ag1", bufs=2) as pool_g,
            tc.tile_pool(name="gch1", bufs=2) as pool_gch,
            tc.tile_pool(name="psA1", bufs=4, space="PSUM") as psA,
            tc.tile_pool(name="psB1", bufs=2, space="PSUM") as psB,
        ):
            qr_sb = pool_qr.tile([P, DB, RANK], F32R)
            nc.sync.dma_start(qr_sb[:], _r(qk_readT).bitcast(F32R))
            qw_sb = pool_qw.tile([P, RB, DHG], F32R)
            nc.sync.dma_start(qw_sb[:], _r(qk_w).bitcast(F32R))
            for sc_i in range(NSC):
                sl = slice(sc_i * SC, (sc_i + 1) * SC)
                xt = pool_x.tile([P, DB, SC], F32R, tag="xt")
                nc.sync.dma_start(xt[:], _r(xT)[:, :, sl].bitcast(F32R))
                gq = pool_g.tile([P, RB, SC], F32, tag="gq")
                nc.sync.dma_start(gq[:], _r(gqT)[:, :, sl])
                gk = pool_g.tile([P, RB, SC], F32, tag="gk")
                nc.sync.dma_start(gk[:], _r(gkT)[:, :, sl])
                qg_ch = pool_gch.tile([P, RB, SC], F32R, tag="qg")
                kg_ch = pool_gch.tile([P, RB, SC], F32R, tag="kg")
                for rb in range(RB):
                    ps = psA.tile([P, SC], F32, tag="tA")
                    for db in range(DB):
                        nc.tensor.matmul(ps[:], qr_sb[:, db, rb * P:(rb + 1) * P],
                                         xt[:, db, :], start=(db == 0),
                                         stop=(db == DB - 1))
                    nc.vector.tensor_tensor(qg_ch[:, rb, :], ps[:], gq[:, rb, :],
                                            ALU.mult)
                    nc.vector.tensor_tensor(kg_ch[:, rb, :], ps[:], gk[:, rb, :],
                                            ALU.mult)
                for db in range(HG):
                    dsl = slice(db * P, (db + 1) * P)
                    psq = psB.tile([P, SC], F32, tag="qB")
                    for rb in range(RB):
                        nc.tensor.matmul(psq[:], qw_sb[:, rb, dsl], qg_ch[:, rb, :],
                                         start=(rb == 0), stop=(rb == RB - 1))
                    nc.scalar.activation(QT_sb[:, db, sl], psq[:], AF.Copy)
                    psk = psB.tile([P, SC], F32, tag="kB")
                    for rb in range(RB):
                        nc.tensor.matmul(psk[:], qw_sb[:, rb, dsl], kg_ch[:, rb, :],
                                         start=(rb == 0), stop=(rb == RB - 1))
                    nc.scalar.activation(KT_sb[:, db, sl], psk[:], AF.Copy)

        # ========== A2+B2 fused per s-chunk: V ==========
        pool_v = ctx.enter_context(tc.tile_pool(name="v", bufs=1))
        V_sb = pool_v.tile([P, SB, DHG], F32R)          # V natural [s, d']
        with (
            tc.tile_pool(name="vread", bufs=1) as pool_vr,
            tc.tile_pool(name="vw", bufs=1) as pool_vw,
            tc.tile_pool(name="ax2", bufs=2) as pool_x,
            tc.tile_pool(name="ag2", bufs=2) as pool_g,
            tc.tile_pool(name="gch2", bufs=2) as pool_gch,
            tc.tile_pool(name="psA2", bufs=4, space="PSUM") as psA,
            tc.tile_pool(name="psB2", bufs=4, space="PSUM") as psB,
        ):
            vr_sb = pool_vr.tile([P, DB, RANK], F32R)
            nc.sync.dma_start(vr_sb[:], _r(v_readT).bitcast(F32R))
            vw_sb = pool_vw.tile([P, RB, DHG], F32R)
            nc.sync.dma_start(vw_sb[:], _r(v_w).bitcast(F32R))
            for sc_i in range(NSC):
                sl = slice(sc_i * SC, (sc_i + 1) * SC)
                xt = pool_x.tile([P, DB, SC], F32R, tag="xt")
                nc.sync.dma_start(xt[:], _r(xT)[:, :, sl].bitcast(F32R))
                gv = pool_g.tile([P, RB, SC], F32, tag="gv")
                nc.sync.dma_start(gv[:], _r(gvT)[:, :, sl])
                vg_ch = pool_gch.tile([P, RB, SC], F32R, tag="vg")
                for rb in range(RB):
                    ps = psA.tile([P, SC], F32, tag="tA")
                    for db in range(DB):
                        nc.tensor.matmul(ps[:], vr_sb[:, db, rb * P:(rb + 1) * P],
                                         xt[:, db, :], start=(db == 0),
                                         stop=(db == DB - 1))
                    nc.vector.tensor_tensor(vg_ch[:, rb, :], ps[:], gv[:, rb, :],
                                            ALU.mult)
                for sj in range(SC // P):
                    s_blk = sc_i * (SC // P) + sj
                    psv = psB.tile([P, DHG], F32, tag="vB")
                    for rb in range(RB):
                        nc.tensor.matmul(psv[:], vg_ch[:, rb, sj * P:(sj + 1) * P],
                                         vw_sb[:, rb, :], start=(rb == 0),
                                         stop=(rb == RB - 1))
                    nc.scalar.activation(V_sb[:, s_blk, :], psv[:], AF.Copy)

        # ========== C + D: attention, AllGather, W_O ==========
        with (
            tc.tile_pool(name="csmall", bufs=1) as pool_c1,
            tc.tile_pool(name="exp", bufs=4) as pool_exp,
            tc.tile_pool(name="rep", bufs=2) as pool_rep,
            tc.tile_pool(name="recip", bufs=2) as pool_recip,
            tc.tile_pool(name="ao", bufs=2) as pool_ao,
            tc.tile_pool(name="wo", bufs=1) as pool_wo,
            tc.tile_pool(name="aof", bufs=3) as pool_aof,
            tc.tile_pool(name="dramb", bufs=5, space="DRAM") as pool_dram,
            tc.tile_pool(name="psC", bufs=2, space="PSUM") as psC,
            tc.tile_pool(name="psC1", bufs=2, space="PSUM") as psC1,
            tc.tile_pool(name="psD", bufs=1, space="PSUM") as psD,
        ):
            # constants
            masks = pool_c1.tile([P, HG, 512], F32)
            nc.vector.memset(masks[:], 1.0)
            for o in range(HG):
                nc.gpsimd.affine_select(
                    out=masks[:, o, :], in_=masks[:, o, :],
                    compare_op=ALU.is_ge, fill=0.0, base=-P * o,
                    pattern=[[1, 512]], channel_multiplier=-1)
            ones_f = pool_c1.tile([P, 1], F32)
            nc.vector.memset(ones_f[:], 0.9 * 0.9)
            ones_r = pool_c1.tile([P, 1], F32R)
            nc.vector.tensor_copy(ones_r[:], ones_f[:])
            onecol = pool_c1.tile([1, P], F32)
            nc.vector.memset(onecol[:], 1.0)
            wo_sb = pool_wo.tile([P, DB, DHG], F32R)
            nc.sync.dma_start(wo_sb[:], _r(wo).bitcast(F32R))

            def head_tail(ao, h, pv, rs):
                """normalize head h: recip(rowsum) -> DMA-replicate -> scale pv."""
                recip = pool_recip.tile([1, 512], F32, tag="recip")
                nc.vector.reciprocal(recip[:], rs[:])
                rep_ps = psD.tile([P, 512], F32, tag="rep")
                nc.tensor.matmul(rep_ps[:], onecol[:], recip[:],
                                 start=True, stop=True)
                rep_sb = pool_rep.tile([P, 512], F32, tag="repsb")
                nc.scalar.activation(rep_sb[:], rep_ps[:], AF.Copy)
                nc.vector.tensor_tensor(ao[:, h, :], pv[:], rep_sb[:], ALU.mult)

            ag_outs = []
            for t in range(NT):
                tsl = slice(t * 512, (t + 1) * 512)
                ao = pool_ao.tile([P, HG, 512], F32R, tag="ao")
                nsj = 4 * (t + 1)
                prev = None
                for h in range(HG):
                    pv = psC.tile([P, 512], F32, tag="pv")
                    rs = psC1.tile([1, 512], F32, tag="rs")
                    for j in range(nsj):
                        jsl = slice(j * P, (j + 1) * P)
                        sc = psC.tile([P, 512], F32, tag="sc")
                        nc.tensor.matmul(sc[:], KT_sb[:, h, jsl],
                                         QT_sb[:, h, tsl], start=True, stop=True)
                        et = pool_exp.tile([P, 512], F32R, tag="et")
                        nc.scalar.activation(et[:], sc[:], AF.Exp,
                                             scale=EXP_SCALE)
                        o = j - 4 * t
                        if o >= 0:
                            nc.vector.tensor_tensor(
                                et[:], et[:], masks[:, o, :].bitcast(F32R),
                                ALU.mult)
                        nc.tensor.matmul(rs[:], ones_r[:], et[:],
                                         start=(j == 0), stop=(j == nsj - 1))
                        nc.tensor.matmul(pv[:], V_sb[:, j, h * P:(h + 1) * P],
                                         et[:], start=(j == 0),
                                         stop=(j == nsj - 1))
                        if j == 0 and prev is not None:
                            head_tail(ao, *prev)   # overlap prior head's tail
                            prev = None
                    prev = (h, pv, rs)
                head_tail(ao, *prev)
                # AllGather this si-chunk across the 4-core group
                bin_t = pool_dram.tile([DHG, 512], F32, tag="bin")
                bout_t = pool_dram.tile([D, 512], F32, tag="bout")
                nc.sync.dma_start(
                    bin_t.rearrange("(h p) s -> p h s", p=P), ao[:].bitcast(F32))
                nc.gpsimd.collective_compute(
                    "AllGather", ALU.bypass, ins=[bin_t[:].opt()],
                    outs=[bout_t[:].opt()], replica_groups=RGROUPS)
                ag_outs.append(bout_t)
            # D: all output chunks emitted after the last AG so D(0..2) fill
            # the final AllGather's latency on PE.
            for t in range(NT):
                bout_t = ag_outs[t]
                for si in range(4):
                    aof = pool_aof.tile([P, DB, P], F32R, tag="aof")
                    nc.sync.dma_start(
                        aof[:],
                        _r(bout_t)[:, :, si * P:(si + 1) * P].bitcast(F32R))
                    ps = psD.tile([P, DHG], F32, tag="d")
                    for dbk in range(DB):
                        nc.tensor.matmul(ps[:], aof[:, dbk, :], wo_sb[:, dbk, :],
                                         start=(dbk == 0), stop=(dbk == DB - 1))
                    o_sb = pool_rep.tile([P, DHG], F32, tag="osb")
                    nc.scalar.activation(o_sb[:], ps[:], AF.Copy)
                    row0 = (t * 4 + si) * P
                    nc.sync.dma_start(out[row0:row0 + P, :], o_sb[:])


def _get_nc():
    if 'nc' not in _CACHE:
        _CACHE['nc'] = _build()
    return _CACHE['nc']


def kernel(**inputs):
    x = np.asarray(inputs["x"], np.float32)
    g_Q = np.asarray(inputs["g_Q"], np.float32)
    g_K = np.asarray(inputs["g_K"], np.float32)
    g_V = np.asarray(inputs["g_V"], np.float32)
    qk_read = np.asarray(inputs["qk_read"], np.float32)
    qk_write = np.asarray(inputs["qk_write"], np.float32)
    v_read = np.asarray(inputs["v_read"], np.float32)
    v_write = np.asarray(inputs["v_write"], np.float32)
    W_O = np.asarray(inputs["W_O"], np.float32)

    nc = _get_nc()
    qk_readT = np.ascontiguousarray(qk_read.T)
    v_readT = np.ascontiguousarray(v_read.T)
    in_maps = []
    for c in range(8):
        b, hg = divmod(c, 4)
        cs = slice(hg * DHG, (hg + 1) * DHG)
        in_maps.append({
            "xT": np.ascontiguousarray(x[b].T),
            "gqT": np.ascontiguousarray(g_Q[b].T),
            "gkT": np.ascontiguousarray(g_K[b].T),
            "gvT": np.ascontiguousarray(g_V[b].T),
            "qk_readT": qk_readT,
            "v_readT": v_readT,
            "qk_write_hg": np.ascontiguousarray(qk_write[:, cs]),
            "v_write_hg": np.ascontiguousarray(v_write[:, cs]),
            "wo_cols": np.ascontiguousarray(W_O[:, cs]),
        })
    res = run_bass_kernel_spmd(nc, in_maps, core_ids=list(range(8)))
    _CACHE['last_results'] = res
    out = np.empty((B, S, D), np.float32)
    for c in range(8):
        b, hg = divmod(c, 4)
        out[b, :, hg * DHG:(hg + 1) * DHG] = res.results[c]["out"]
    return out



# revision 11
# speedup vs baseline: 1.0866x; 1.0866x over previous
"""Distributed Bass kernel for nn_AttentionCircuit (B=2,S=2048,D=2048,RANK=512,H=16).

Sharding: 8 cores = 2 batches x 4 group-positions. Within a 4-core batch
group, core g owns s-chunk g (512 rows) for the A/B phases and head-group g
(4 heads / 512 D-cols) for attention. All matmul operands bf16 (1 cyc/row on
PE, half the DMA/collective bytes of fp32; validated rel-err ~5e-3).

Per-core dataflow:
  A (own s-chunk): t^T = read @ x^T, gate with g^T  -> tqg/tkg/tvg [rank, s]
  B (own s-chunk, own cols): Q^T/K^T [cols, s], V [s, cols]
  AG: single packed AllGather of (Q^T, K^T, V) chunks -> full-S tensors
  C per 512-wide t-chunk, per head: scores^T = K^T.T Q^T -> exp (pair-batched
     on Act engine) -> causal mask (block-skip + static masks on last 2
     pairs) -> rowsum via ones-matmul of DVE pair-sums -> PV matmul ->
     normalize with outer(1/0.81, 1/rowsum) bcast matmul -> ao [dh, i] bf16
  D per t-chunk (no AG needed): partial out rows = ao^T.T @ W_O[own rows,:]
     -> ReduceScatter(add) over the group scatters rows to cores.
Host reassembles: core (b,g) holds rows t*512+g*128..+128 of batch b.
"""
import sys
import numpy as np
import ml_dtypes

sys.path.insert(0, '/opt/trn_rl_repo')

import concourse.bass as bass  # noqa: E402
from concourse import bacc  # noqa: E402
import concourse.mybir as mybir  # noqa: E402
import concourse.tile as tile  # noqa: E402
from concourse.bass_utils import run_bass_kernel_spmd  # noqa: E402

B, S, D = 2, 2048, 2048
RANK = 512
NH = 16
HG = 4              # heads per core / group size
DHG = D // HG       # 512 cols per core
P = 128
DB = D // P         # 16 d-blocks
RB = RANK // P      # 4 rank-blocks (== own-col blocks)
SC = S // 4         # 512: own s-chunk width == t-chunk width
NT = S // SC        # 4 t-chunks

F32 = mybir.dt.float32
BF = mybir.dt.bfloat16
AF = mybir.ActivationFunctionType
ALU = mybir.AluOpType

EXP_SCALE = 1.0 / float(np.sqrt(P))
INV_KEEP2 = 1.0 / (0.9 * 0.9)
RGROUPS = [[0, 1, 2, 3], [4, 5, 6, 7]]

_CACHE = {}


def _r(ap):
    """[ (o p), f ] DRAM tensor -> [p, o, f] partition-tiled view."""
    return ap.rearrange("(o p) f -> p o f", p=P)


def _build():
    nc = bacc.Bacc("TRN2", target_bir_lowering=False, debug=False,
                   enable_asserts=False, num_devices=8)
    xT = nc.dram_tensor("xT", [D, SC], BF, kind="ExternalInput").ap()
    gqT = nc.dram_tensor("gqT", [RANK, SC], BF, kind="ExternalInput").ap()
    gkT = nc.dram_tensor("gkT", [RANK, SC], BF, kind="ExternalInput").ap()
    gvT = nc.dram_tensor("gvT", [RANK, SC], BF, kind="ExternalInput").ap()
    qk_readT = nc.dram_tensor("qk_readT", [D, RANK], BF, kind="ExternalInput").ap()
    v_readT = nc.dram_tensor("v_readT", [D, RANK], BF, kind="ExternalInput").ap()
    qk_w = nc.dram_tensor("qk_w", [RANK, DHG], BF, kind="ExternalInput").ap()
    v_w = nc.dram_tensor("v_w", [RANK, DHG], BF, kind="ExternalInput").ap()
    wo_rows = nc.dram_tensor("wo_rows", [DHG, D], BF, kind="ExternalInput").ap()
    out = nc.dram_tensor("out", [NT, P, D], BF, kind="ExternalOutput").ap()

    with tile.TileContext(nc) as tc:
        _body(tc, xT, gqT, gkT, gvT, qk_readT, v_readT, qk_w, v_w, wo_rows, out)
    nc.compile()
    return nc


def _body(tc, xT, gqT, gkT, gvT, qk_readT, v_readT, qk_w, v_w, wo_rows, out):
    nc = tc.nc
    import contextlib
    ctx = contextlib.ExitStack()
    with ctx:
        pool_main = ctx.enter_context(tc.tile_pool(name="main", bufs=1))
        pool_ao = ctx.enter_context(tc.tile_pool(name="ao", bufs=2))
        pool_et = ctx.enter_context(tc.tile_pool(name="et", bufs=3))
        pool_ets = ctx.enter_context(tc.tile_pool(name="ets", bufs=2))
        pool_sm = ctx.enter_context(tc.tile_pool(name="sm", bufs=2))
        pool_rsin = ctx.enter_context(tc.tile_pool(name="rsin", bufs=2))
        pool_dram = ctx.enter_context(tc.tile_pool(name="dramb", bufs=1,
                                                   space="DRAM"))

        # ---- long-lived tensors / constants
        QT_sb = pool_main.tile([P, HG, NT, SC], BF)   # Q^T [dh, head, chunk, s]
        KT_sb = pool_main.tile([P, HG, NT, SC], BF)
        V_sb = pool_main.tile([P, DB, DHG], BF)       # V [s-block, own cols]
        wo_sb = pool_main.tile([P, RB, D], BF)        # W_O own rows
        masks = pool_main.tile([P, HG, SC], BF)
        ones_r = pool_main.tile([P, 1], BF)
        onecol = pool_main.tile([1, P], BF)

        nc.sync.dma_start(wo_sb[:], _r(wo_rows))
        nc.vector.memset(masks[:], 1.0)
        for o in range(HG):
            nc.gpsimd.affine_select(
                out=masks[:, o, :], in_=masks[:, o, :],
                compare_op=ALU.is_ge, fill=0.0, base=-P * o,
                pattern=[[1, SC]], channel_multiplier=-1)
        nc.vector.memset(ones_r[:], 1.0)
        nc.vector.memset(onecol[:], INV_KEEP2)

        qkv_in = pool_dram.tile([3, DHG, SC], BF)
        qkv_out = pool_dram.tile([HG, 3, DHG, SC], BF)
        rs_in = pool_dram.tile([NT, SC, D], BF)
        rout = pool_dram.tile([NT, P, D], BF)

        # ========== A on own s-chunk: gated low-rank t (rank x s) ==========
        with (
            tc.tile_pool(name="ab", bufs=1) as pab,
            tc.tile_pool(name="psA", bufs=2, space="PSUM") as psA,
        ):
            xt = pab.tile([P, DB, SC], BF)
            nc.sync.dma_start(xt[:], _r(xT))
            gq = pab.tile([P, RB, SC], BF)
            nc.sync.dma_start(gq[:], _r(gqT))
            gk = pab.tile([P, RB, SC], BF)
            nc.sync.dma_start(gk[:], _r(gkT))
            qr = pab.tile([P, DB, RANK], BF)
            nc.sync.dma_start(qr[:], _r(qk_readT))
            vr = pab.tile([P, DB, RANK], BF)
            nc.sync.dma_start(vr[:], _r(v_readT))
            gv = pab.tile([P, RB, SC], BF)
            nc.sync.dma_start(gv[:], _r(gvT))

            tqg = pab.tile([P, RB, SC], BF)
            tkg = pab.tile([P, RB, SC], BF)
            tvg = pab.tile([P, RB, SC], BF)

            for rb in range(RB):
                ps = psA.tile([P, SC], F32, tag="a")
                for db in range(DB):
                    nc.tensor.matmul(ps[:], qr[:, db, rb * P:(rb + 1) * P],
                                     xt[:, db, :], start=(db == 0),
                                     stop=(db == DB - 1))
                nc.vector.tensor_tensor(tqg[:, rb, :], ps[:], gq[:, rb, :],
                                        ALU.mult)
                nc.vector.tensor_tensor(tkg[:, rb, :], ps[:], gk[:, rb, :],
                                        ALU.mult)
            for rb in range(RB):
                ps = psA.tile([P, SC], F32, tag="a")
                for db in range(DB):
                    nc.tensor.matmul(ps[:], vr[:, db, rb * P:(rb + 1) * P],
                                     xt[:, db, :], start=(db == 0),
                                     stop=(db == DB - 1))
                nc.vector.tensor_tensor(tvg[:, rb, :], ps[:], gv[:, rb, :],
                                        ALU.mult)

            nc.sync.dma_start(_r(qkv_in[0]), tqg[:])
            nc.sync.dma_start(_r(qkv_in[1]), tkg[:])
            nc.sync.dma_start(_r(qkv_in[2]), tvg[:])

        # ========== AllGather gated-t across the 4-core group ==========
        nc.gpsimd.collective_compute(
            "AllGather", ALU.bypass, ins=[qkv_in[:].opt()],
            outs=[qkv_out[:].opt()], replica_groups=RGROUPS)

        pb = ctx.enter_context(tc.tile_pool(name="bpool", bufs=1))
        qw = pb.tile([P, RB, DHG], BF)
        nc.sync.dma_start(qw[:], _r(qk_w))
        vw = pb.tile([P, RB, DHG], BF)
        nc.sync.dma_start(vw[:], _r(v_w))
        tqgf = pb.tile([P, RB, S], BF)
        tkgf = pb.tile([P, RB, S], BF)
        tvgf = pb.tile([P, RB, S], BF)
        for c in range(HG):
            csl = slice(c * SC, (c + 1) * SC)
            nc.sync.dma_start(tqgf[:, :, csl], _r(qkv_out[c, 0]))
            nc.sync.dma_start(tkgf[:, :, csl], _r(qkv_out[c, 1]))
            nc.sync.dma_start(tvgf[:, :, csl], _r(qkv_out[c, 2]))

        psB = ctx.enter_context(tc.tile_pool(name="psB", bufs=2, space="PSUM"))
        psSC = ctx.enter_context(tc.tile_pool(name="psSC", bufs=2, space="PSUM"))
        psPV = ctx.enter_context(tc.tile_pool(name="psPV", bufs=1, space="PSUM"))
        psRS = ctx.enter_context(tc.tile_pool(name="psRS", bufs=1, space="PSUM"))
        psREP = ctx.enter_context(tc.tile_pool(name="psREP", bufs=1, space="PSUM"))

        # ========== B(chunk) interleaved with C (attention) + D + RS ==========
        for t in range(NT):
            # B for s-chunk t: Q^T/K^T [own cols, chunk], V [chunk, own cols]
            csl = slice(t * SC, (t + 1) * SC)
            for db in range(RB):
                dsl = slice(db * P, (db + 1) * P)
                psq = psB.tile([P, SC], F32, tag="b")
                for rb in range(RB):
                    nc.tensor.matmul(psq[:], qw[:, rb, dsl], tqgf[:, rb, csl],
                                     start=(rb == 0), stop=(rb == RB - 1))
                nc.scalar.activation(QT_sb[:, db, t, :], psq[:], AF.Copy)
                psk = psB.tile([P, SC], F32, tag="b")
                for rb in range(RB):
                    nc.tensor.matmul(psk[:], qw[:, rb, dsl], tkgf[:, rb, csl],
                                     start=(rb == 0), stop=(rb == RB - 1))
                nc.scalar.activation(KT_sb[:, db, t, :], psk[:], AF.Copy)
            for sj in range(RB):
                sb = t * RB + sj
                ssl2 = slice(sb * P, (sb + 1) * P)
                psv = psB.tile([P, DHG], F32, tag="b")
                for rb in range(RB):
                    nc.tensor.matmul(psv[:], tvgf[:, rb, ssl2], vw[:, rb, :],
                                     start=(rb == 0), stop=(rb == RB - 1))
                nc.scalar.activation(V_sb[:, sb, :], psv[:], AF.Copy)
            ao = pool_ao.tile([P, HG, SC], BF, tag="ao")
            npair = 2 * (t + 1)
            for h in range(HG):
                hsl = slice(h * P, (h + 1) * P)
                pv = psPV.tile([P, SC], F32, tag="pv")
                rs = psRS.tile([1, SC], F32, tag="rs")
                for q in range(npair):
                    etps = []
                    for k in range(2):
                        jb = 2 * q + k
                        jc, jp = divmod(jb, RB)
                        sc = psSC.tile([P, SC], F32, tag="sc")
                        nc.tensor.matmul(
                            sc[:], KT_sb[:, h, jc, jp * P:(jp + 1) * P],
                            QT_sb[:, h, t, :], start=True, stop=True)
                        etp = pool_et.tile([P, SC], BF, tag="et", bufs=4)
                        nc.scalar.activation(etp[:], sc[:], AF.Exp,
                                             scale=EXP_SCALE)
                        o = jb - 4 * t
                        if o >= 0:
                            nc.vector.tensor_tensor(etp[:], etp[:],
                                                    masks[:, o, :], ALU.mult)
                        nc.tensor.matmul(pv[:], V_sb[:, jb, hsl], etp[:],
                                         start=(q == 0 and k == 0),
                                         stop=(q == npair - 1 and k == 1))
                        etps.append(etp)
                    ets = pool_ets.tile([P, SC], BF, tag="ets")
                    nc.vector.tensor_tensor(ets[:], etps[0][:], etps[1][:],
                                            ALU.add)
                    nc.tensor.matmul(rs[:], ones_r[:], ets[:],
                                     start=(q == 0), stop=(q == npair - 1))
                recip = pool_sm.tile([1, SC], BF, tag="recip")
                with nc.allow_low_precision(reason="bf16 1/Z validated offline"):
                    nc.vector.reciprocal(recip[:], rs[:])
                rep = psREP.tile([P, SC], F32, tag="rep")
                nc.tensor.matmul(rep[:], onecol[:], recip[:],
                                 start=True, stop=True)
                nc.scalar.activation(ao[:, h, :], pv[:], AF.Copy)
                nc.vector.tensor_tensor(ao[:, h, :], ao[:, h, :], rep[:],
                                        ALU.mult)
            # D: partial out rows for this t-chunk (local ao only)
            for isub in range(4):
                rsin_sb = pool_rsin.tile([P, HG, SC], BF, tag="rsin")
                for oc in range(4):
                    psd = psB.tile([P, SC], F32, tag="b")
                    for h in range(HG):
                        nc.tensor.matmul(
                            psd[:], ao[:, h, isub * P:(isub + 1) * P],
                            wo_sb[:, h, oc * SC:(oc + 1) * SC],
                            start=(h == 0), stop=(h == HG - 1))
                    nc.vector.tensor_copy(rsin_sb[:, oc, :], psd[:])
                nc.sync.dma_start(
                    _r(rs_in[t])[:, isub, :],
                    rsin_sb[:].rearrange("p o f -> p (o f)"))
            nc.gpsimd.collective_compute(
                "ReduceScatter", ALU.add, ins=[rs_in[t].opt()],
                outs=[rout[t].opt()], replica_groups=RGROUPS)
            nc.sync.dma_start(out[t], rout[t])


def _get_nc():
    if 'nc' not in _CACHE:
        _CACHE['nc'] = _build()
    return _CACHE['nc']


def _bf(a):
    return np.ascontiguousarray(np.asarray(a, np.float32)).astype(
        ml_dtypes.bfloat16)


def kernel(**inputs):
    x = np.asarray(inputs["x"], np.float32)
    g_Q = np.asarray(inputs["g_Q"], np.float32)
    g_K = np.asarray(inputs["g_K"], np.float32)
    g_V = np.asarray(inputs["g_V"], np.float32)
    qk_read = np.asarray(inputs["qk_read"], np.float32)
    qk_write = np.asarray(inputs["qk_write"], np.float32)
    v_read = np.asarray(inputs["v_read"], np.float32)
    v_write = np.asarray(inputs["v_write"], np.float32)
    W_O = np.asarray(inputs["W_O"], np.float32)

    nc = _get_nc()
    qk_readT = _bf(qk_read.T)
    v_readT = _bf(v_read.T)
    in_maps = []
    for c in range(8):
        b, g = divmod(c, 4)
        ssl = slice(g * SC, (g + 1) * SC)
        in_maps.append({
            "xT": _bf(x[b].T[:, ssl]),
            "gqT": _bf(g_Q[b].T[:, ssl]),
            "gkT": _bf(g_K[b].T[:, ssl]),
            "gvT": _bf(g_V[b].T[:, ssl]),
            "qk_readT": qk_readT,
            "v_readT": v_readT,
            "qk_w": _bf(qk_write[:, ssl]),
            "v_w": _bf(v_write[:, ssl]),
            "wo_rows": _bf(W_O[ssl, :]),
        })
    res = run_bass_kernel_spmd(nc, in_maps, core_ids=list(range(8)))
    _CACHE['last_results'] = res
    out = np.empty((B, S, D), np.float32)
    for c in range(8):
        b, g = divmod(c, 4)
        o = np.asarray(res.results[c]["out"], dtype=ml_dtypes.bfloat16)
        for t in range(NT):
            r0 = t * SC + g * P
            out[b, r0:r0 + P, :] = o[t].astype(np.float32)
    return out
